# revision 1
# baseline (speedup 1.0000x reference)
"""Chamfer distance kernel for Trainium2 (8 NeuronCores, batch-parallel).

Strategy
--------
B=8 batches, one per core (SPMD: same program, per-core data).
Per core (N=M=8192, 3-D points):
  d[n,m] = |x1_n|^2 + |x2_m|^2 - 2 x1_n.x2_m  is computed fully inside
  PSUM by a single K=5 matmul with homogeneous coordinates:
    lhsT rows = [-2x, -2y, -2z, 1, n1],  rhs rows = [x', y', z', n2', 1]
  Two symmetric passes (rows = x1 points, then rows = x2 points) give the
  row-direction min+argmin for both outputs.  Per 128-row block the ACT
  engine moves PSUM->SBUF, the DVE does one reduce_min over [128, 8192]
  and one max_index (equality matcher, first occurrence == np.argmin
  tie-break) to extract the argmin.
Outputs come back as [128, 64] tiles (partition-major), the host
transpose-flattens them.
"""

import numpy as np

import concourse.bacc as bacc
import concourse.bass as bass
import concourse.mybir as mybir
from concourse import tile
from concourse.bass_utils import run_bass_kernel_spmd

F32 = mybir.dt.float32
I32 = mybir.dt.int32
U32 = mybir.dt.uint32

_PROGRAM_CACHE = {}


def _emit_pass(nc, lhsU, rhsU, base, dtile, itile, rowbuf_pool, psum_pool,
               scratch_pool, n_pts, m_pts, mm_dt):
    """One direction: for each 128-row block of lhs points, min+argmin over
    all m_pts columns.  The lhsT role slice is lhsU[base:base+5] =
    [-2x, -2y, -2z, 1, n]; the rhs role slice is rhsU[base:base+5] =
    [x, y, z, n, 1] (matmul requires equal operand base partitions)."""
    n_blocks = n_pts // 128
    n_groups = m_pts // 2048

    for nb in range(n_blocks):
        rowbuf = rowbuf_pool.tile([128, m_pts], F32, tag="rowbuf")
        lhs_ap = lhsU[base:base + 5, nb * 128:(nb + 1) * 128]
        if mm_dt is not F32:
            lhs_ap = lhs_ap.bitcast(mm_dt)
        for g in range(n_groups):
            psum = psum_pool.tile([128, 2048], F32, tag="psum")
            for q in range(4):
                rhs_ap = rhsU[base:base + 5, (g * 4 + q) * 512:(g * 4 + q + 1) * 512]
                if mm_dt is not F32:
                    rhs_ap = rhs_ap.bitcast(mm_dt)
                nc.tensor.matmul(
                    psum[:, q * 512:(q + 1) * 512],
                    lhs_ap,
                    rhs_ap,
                    start=True, stop=True,
                )
            nc.scalar.activation(
                rowbuf[:, g * 2048:(g + 1) * 2048], psum[:],
                mybir.ActivationFunctionType.Copy,
            )
        # row min -> dist column
        nc.vector.tensor_reduce(
            dtile[:, nb:nb + 1], rowbuf[:],
            axis=mybir.AxisListType.X, op=mybir.AluOpType.min,
        )
        # match the min value back to its first position
        q8 = scratch_pool.tile([128, 8], F32, tag="q8")
        ix = scratch_pool.tile([128, 8], U32, tag="ix")
        nc.vector.tensor_copy(q8[:], dtile[:, nb:nb + 1].broadcast_to((128, 8)))
        nc.vector.max_index(ix[:], q8[:], rowbuf[:])
        nc.vector.tensor_copy(itile[:, nb:nb + 1], ix[:, 0:1])


def _build_program(n_pts=8192, m_pts=8192, n_cores=8, mm_dtype="f32", repeat=1):
    key = (n_pts, m_pts, n_cores, mm_dtype, repeat)
    if key in _PROGRAM_CACHE:
        return _PROGRAM_CACHE[key]

    mm_dt = {"f32": F32, "f32r": mybir.dt.float32r}[mm_dtype]

    nc = bacc.Bacc("TRN2", target_bir_lowering=False, debug=False,
                   num_devices=n_cores)
    # rows [x, y, z, ones]; the ones row seeds the homogeneous-coordinate rows
    c1 = nc.dram_tensor("c1", [4, n_pts], F32, kind="ExternalInput")
    c2 = nc.dram_tensor("c2", [4, m_pts], F32, kind="ExternalInput")
    d1 = nc.dram_tensor("d1", [128, n_pts // 128], F32, kind="ExternalOutput")
    i1 = nc.dram_tensor("i1", [128, n_pts // 128], I32, kind="ExternalOutput")
    d2 = nc.dram_tensor("d2", [128, m_pts // 128], F32, kind="ExternalOutput")
    i2 = nc.dram_tensor("i2", [128, m_pts // 128], I32, kind="ExternalOutput")

    with tile.TileContext(nc) as tc:
        with tc.tile_pool(name="persist", bufs=1) as persist, \
             tc.tile_pool(name="finals", bufs=1) as finals:
            # Combined tiles.  Matmul needs BOTH operands at the same base
            # partition (0/32/64), so:
            #   U1: lhsT-form (A = [-2x,-2y,-2z, 1, n1]) at rows 0-4,
            #       rhs-form  (B = [x, y, z, n1, 1])     at rows 32-36
            #   U2: rhs-form  (B = [x', y', z', n2', 1]) at rows 0-4,
            #       lhsT-form (A = [-2x',-2y',-2z', 1, n2]) at rows 32-36
            # Pass A pairs U1[0:5] x U2[0:5]; pass B pairs U2[32:37] x U1[32:37].
            # One 32KB column range per tensor covers both forms.
            U1 = persist.tile([37, n_pts], F32, tag="U1")
            U2 = persist.tile([37, m_pts], F32, tag="U2")
            # ones column for the norm matmuls; [35, 1] so both base-0 and
            # base-32 slices exist (matmul operands must share their base)
            ones_col = persist.tile([35, 1], F32, tag="ones_col")
            nc.vector.memset(ones_col[:], 1.0)

            d1t = finals.tile([128, n_pts // 128], F32, tag="d1t")
            i1t = finals.tile([128, n_pts // 128], I32, tag="i1t")
            d2t = finals.tile([128, m_pts // 128], F32, tag="d2t")
            i2t = finals.tile([128, m_pts // 128], I32, tag="i2t")

            # ---- prep both tensors ----
            with tc.tile_pool(name="prep", bufs=1) as prep, \
                 tc.tile_pool(name="preppsum", bufs=2, space="PSUM") as ppsum:
                # a = base row of the A-form, b = base row of the B-form.
                # Engine ops keep all APs at one base (partition quadrant
                # rule); DMA moves rows across bases.
                for U, c, npts, a, b in ((U1, c1, n_pts, 0, 32),
                                         (U2, c2, m_pts, 32, 0)):
                    nc.sync.dma_start(U[b:b + 3, :], c.ap()[0:3, :])   # B coords
                    nc.sync.dma_start(U[a + 3:a + 4, :], c.ap()[3:4, :])  # A ones
                    nc.sync.dma_start(U[b + 4:b + 5, :], c.ap()[3:4, :])  # B ones
                    sq = prep.tile([35, npts], F32, tag="sq")
                    nrow = prep.tile([1, npts], F32, tag="nrow")
                    nc.scalar.activation(sq[b:b + 3, :], U[b:b + 3, :],
                                         mybir.ActivationFunctionType.Square)
                    for cchunk in range(npts // 512):
                        ps = ppsum.tile([1, 512], F32, tag="ps")
                        nc.tensor.matmul(ps[:], ones_col[b:b + 3, :],
                                         sq[b:b + 3, cchunk * 512:(cchunk + 1) * 512],
                                         start=True, stop=True)
                        nc.scalar.activation(
                            nrow[:, cchunk * 512:(cchunk + 1) * 512],
                            ps[:], mybir.ActivationFunctionType.Copy)
                    nc.sync.dma_start(U[b + 3:b + 4, :], nrow[:])  # B n-row
                    nc.sync.dma_start(U[a + 4:a + 5, :], nrow[:])  # A n-row
                    # A coords = -2 * B coords: scale in place (same base),
                    # then DMA into the A rows
                    nc.vector.tensor_scalar(
                        out=sq[b:b + 3, :], in0=U[b:b + 3, :],
                        scalar1=-2.0, scalar2=None,
                        op0=mybir.AluOpType.mult)
                    nc.sync.dma_start(U[a:a + 3, :], sq[b:b + 3, :])

            # ---- main passes ----
            with tc.tile_pool(name="rowbuf", bufs=2) as rowbuf_pool, \
                 tc.tile_pool(name="mainpsum", bufs=2, space="PSUM") as psum_pool, \
                 tc.tile_pool(name="scratch", bufs=2) as scratch_pool:
                for _ in range(repeat):
                    _emit_pass(nc, U1, U2, 0, d1t, i1t, rowbuf_pool, psum_pool,
                               scratch_pool, n_pts, m_pts, mm_dt)
                    _emit_pass(nc, U2, U1, 32, d2t, i2t, rowbuf_pool, psum_pool,
                               scratch_pool, m_pts, n_pts, mm_dt)

            # clamp tiny negative rounding like the reference's max(d, 0)
            nc.scalar.activation(d1t[:], d1t[:], mybir.ActivationFunctionType.Relu)
            nc.scalar.activation(d2t[:], d2t[:], mybir.ActivationFunctionType.Relu)
            nc.sync.dma_start(d1.ap(), d1t[:])
            nc.sync.dma_start(i1.ap(), i1t[:])
            nc.sync.dma_start(d2.ap(), d2t[:])
            nc.sync.dma_start(i2.ap(), i2t[:])

    nc.compile()
    _PROGRAM_CACHE[key] = nc
    return nc


def kernel(xyz1: np.ndarray, xyz2: np.ndarray, mm_dtype: str = "f32",
           repeat: int = 1, _return_results_only: bool = False):
    xyz1 = np.asarray(xyz1, dtype=np.float32)
    xyz2 = np.asarray(xyz2, dtype=np.float32)
    B, N, _ = xyz1.shape
    _, M, _ = xyz2.shape
    assert B == 8 and N == 8192 and M == 8192, (B, N, M)

    nc = _build_program(N, M, B, mm_dtype, repeat)

    ones_n = np.ones((1, N), np.float32)
    ones_m = np.ones((1, M), np.float32)
    in_maps = [
        {"c1": np.concatenate([np.ascontiguousarray(xyz1[b].T), ones_n]),
         "c2": np.concatenate([np.ascontiguousarray(xyz2[b].T), ones_m])}
        for b in range(B)
    ]
    res = run_bass_kernel_spmd(nc, in_maps, list(range(B)))

    dist1 = np.empty((B, N), np.float32)
    dist2 = np.empty((B, M), np.float32)
    idx1 = np.empty((B, N), np.int32)
    idx2 = np.empty((B, M), np.int32)
    for b in range(B):
        r = res.results[b]
        dist1[b] = np.asarray(r["d1"]).T.reshape(-1)
        idx1[b] = np.asarray(r["i1"]).T.reshape(-1)
        dist2[b] = np.asarray(r["d2"]).T.reshape(-1)
        idx2[b] = np.asarray(r["i2"]).T.reshape(-1)
    return dist1, dist2, idx1, idx2



# revision 6
# speedup vs baseline: 78.2175x; 78.2175x over previous
"""Chamfer distance kernel for Trainium2 (8 NeuronCores, batch-parallel).

Strategy
--------
B=8 batches, one per core (SPMD: same program, per-core data).
Per core (N=M=8192, 3-D points):
  d[n,m] = |x1_n|^2 + |x2_m|^2 - 2 x1_n.x2_m  is computed fully inside
  PSUM by a single K=5 matmul with homogeneous coordinates:
    lhsT rows = [-2x, -2y, -2z, 1, n1],  rhs rows = [x', y', z', n2', 1]
  Two symmetric passes (rows = x1 points, then rows = x2 points) give the
  row-direction min+argmin for both outputs.  Per 128-row block the ACT
  engine moves PSUM->SBUF, the DVE does one reduce_min over [128, 8192]
  and one max_index (equality matcher, first occurrence == np.argmin
  tie-break) to extract the argmin.
Outputs come back as [128, 64] tiles (partition-major), the host
transpose-flattens them.
"""

import numpy as np

import concourse.bacc as bacc
import concourse.bass as bass
import concourse.mybir as mybir
from concourse import tile
from concourse.bass_utils import run_bass_kernel_spmd

F32 = mybir.dt.float32
I32 = mybir.dt.int32
U32 = mybir.dt.uint32

_PROGRAM_CACHE = {}


def _emit_pass(nc, lhsU, rhsU, base, dtile, itile, rowbuf_pool, psum_pool,
               scratch_pool, n_pts, m_pts, mm_dt, pool_folds=True):
    """One direction: for each 128-row block of lhs points, min+argmin over
    all m_pts columns.  The lhsT role slice is lhsU[base:base+5] =
    [-2x, -2y, -2z, 1, n]; the rhs role slice is rhsU[base:base+5] =
    [x, y, z, n, 1] (matmul requires equal operand base partitions).

    Engine split per block: PE fills PSUM groups (f32r), ACT drains them to
    rowbuf, the Pool engine folds each copied group 2048->512 with two
    elementwise mins, DVE reduces the folded [128, 2048] to the row min and
    runs one full-row max_index for the exact first-occurrence argmin."""
    n_blocks = n_pts // 128
    n_groups = m_pts // 2048

    for nb in range(n_blocks):
        rowbuf = rowbuf_pool.tile([128, m_pts], F32, tag="rowbuf")
        if pool_folds:
            fold1 = scratch_pool.tile([128, 1024], F32, tag="fold1")
            fold2 = scratch_pool.tile([128, n_groups * 512], F32, tag="fold2")
        else:
            gmin = scratch_pool.tile([128, n_groups], F32, tag="gmin")
        lhs_ap = lhsU[base:base + 5, nb * 128:(nb + 1) * 128]
        if mm_dt is not F32:
            lhs_ap = lhs_ap.bitcast(mm_dt)
        for g in range(n_groups):
            psum = psum_pool.tile([128, 2048], F32, tag="psum")
            for q in range(4):
                rhs_ap = rhsU[base:base + 5, (g * 4 + q) * 512:(g * 4 + q + 1) * 512]
                if mm_dt is not F32:
                    rhs_ap = rhs_ap.bitcast(mm_dt)
                nc.tensor.matmul(
                    psum[:, q * 512:(q + 1) * 512],
                    lhs_ap,
                    rhs_ap,
                    start=True, stop=True,
                )
            nc.scalar.activation(
                rowbuf[:, g * 2048:(g + 1) * 2048], psum[:],
                mybir.ActivationFunctionType.Copy,
            )
            if pool_folds:
                # fold this group's copy 2048 -> 1024 -> 512 on the Pool
                # engine so DVE only reduces the folded remainder
                rb = rowbuf[:, g * 2048:(g + 1) * 2048]
                nc.gpsimd.tensor_tensor(
                    fold1[:], rb[:, 0:1024], rb[:, 1024:2048],
                    mybir.AluOpType.min)
                nc.gpsimd.tensor_tensor(
                    fold2[:, g * 512:(g + 1) * 512],
                    fold1[:, 0:512], fold1[:, 512:1024],
                    mybir.AluOpType.min)
            else:
                # value scan straight out of PSUM, decoupled from the ACT copy
                nc.vector.tensor_reduce(
                    gmin[:, g:g + 1], psum[:],
                    axis=mybir.AxisListType.X, op=mybir.AluOpType.min,
                )
        # row min -> dist column
        nc.vector.tensor_reduce(
            dtile[:, nb:nb + 1], fold2[:] if pool_folds else gmin[:],
            axis=mybir.AxisListType.X, op=mybir.AluOpType.min,
        )
        # match the min value back to its first position
        q8 = scratch_pool.tile([128, 8], F32, tag="q8")
        ix = scratch_pool.tile([128, 8], U32, tag="ix")
        nc.vector.tensor_copy(q8[:], dtile[:, nb:nb + 1].broadcast_to((128, 8)))
        nc.vector.max_index(ix[:], q8[:], rowbuf[:])
        nc.vector.tensor_copy(itile[:, nb:nb + 1], ix[:, 0:1])


def _build_program(n_pts=8192, m_pts=8192, n_cores=8, mm_dtype="f32r", repeat=1,
                   pool_folds=True):
    key = (n_pts, m_pts, n_cores, mm_dtype, repeat, pool_folds)
    if key in _PROGRAM_CACHE:
        return _PROGRAM_CACHE[key]

    mm_dt = {"f32": F32, "f32r": mybir.dt.float32r}[mm_dtype]

    nc = bacc.Bacc("TRN2", target_bir_lowering=False, debug=False,
                   num_devices=n_cores)
    # rows [x, y, z, ones]; the ones row seeds the homogeneous-coordinate rows
    c1 = nc.dram_tensor("c1", [4, n_pts], F32, kind="ExternalInput")
    c2 = nc.dram_tensor("c2", [4, m_pts], F32, kind="ExternalInput")
    d1 = nc.dram_tensor("d1", [128, n_pts // 128], F32, kind="ExternalOutput")
    i1 = nc.dram_tensor("i1", [128, n_pts // 128], I32, kind="ExternalOutput")
    d2 = nc.dram_tensor("d2", [128, m_pts // 128], F32, kind="ExternalOutput")
    i2 = nc.dram_tensor("i2", [128, m_pts // 128], I32, kind="ExternalOutput")

    with tile.TileContext(nc) as tc:
        with tc.tile_pool(name="persist", bufs=1) as persist, \
             tc.tile_pool(name="finals", bufs=1) as finals:
            # Combined tiles.  Matmul needs BOTH operands at the same base
            # partition (0/32/64), so:
            #   U1: lhsT-form (A = [-2x,-2y,-2z, 1, n1]) at rows 0-4,
            #       rhs-form  (B = [x, y, z, n1, 1])     at rows 32-36
            #   U2: rhs-form  (B = [x', y', z', n2', 1]) at rows 0-4,
            #       lhsT-form (A = [-2x',-2y',-2z', 1, n2]) at rows 32-36
            # Pass A pairs U1[0:5] x U2[0:5]; pass B pairs U2[32:37] x U1[32:37].
            # One 32KB column range per tensor covers both forms.
            U1 = persist.tile([37, n_pts], F32, tag="U1")
            U2 = persist.tile([37, m_pts], F32, tag="U2")
            # ones column for the norm matmuls; [35, 1] so both base-0 and
            # base-32 slices exist (matmul operands must share their base)
            ones_col = persist.tile([35, 1], F32, tag="ones_col")
            nc.vector.memset(ones_col[:], 1.0)

            d1t = finals.tile([128, n_pts // 128], F32, tag="d1t")
            i1t = finals.tile([128, n_pts // 128], I32, tag="i1t")
            d2t = finals.tile([128, m_pts // 128], F32, tag="d2t")
            i2t = finals.tile([128, m_pts // 128], I32, tag="i2t")

            # ---- prep both tensors ----
            with tc.tile_pool(name="prep", bufs=1) as prep, \
                 tc.tile_pool(name="preppsum", bufs=2, space="PSUM") as ppsum:
                # a = base row of the A-form, b = base row of the B-form.
                # Engine ops keep all APs at one base (partition quadrant
                # rule); DMA moves rows across bases.
                for U, c, npts, a, b in ((U1, c1, n_pts, 0, 32),
                                         (U2, c2, m_pts, 32, 0)):
                    nc.sync.dma_start(U[b:b + 3, :], c.ap()[0:3, :])   # B coords
                    nc.sync.dma_start(U[a + 3:a + 4, :], c.ap()[3:4, :])  # A ones
                    nc.sync.dma_start(U[b + 4:b + 5, :], c.ap()[3:4, :])  # B ones
                    sq = prep.tile([35, npts], F32, tag="sq")
                    nrow = prep.tile([1, npts], F32, tag="nrow")
                    nc.scalar.activation(sq[b:b + 3, :], U[b:b + 3, :],
                                         mybir.ActivationFunctionType.Square)
                    for cchunk in range(npts // 512):
                        ps = ppsum.tile([1, 512], F32, tag="ps")
                        nc.tensor.matmul(ps[:], ones_col[b:b + 3, :],
                                         sq[b:b + 3, cchunk * 512:(cchunk + 1) * 512],
                                         start=True, stop=True)
                        nc.scalar.activation(
                            nrow[:, cchunk * 512:(cchunk + 1) * 512],
                            ps[:], mybir.ActivationFunctionType.Copy)
                    nc.sync.dma_start(U[b + 3:b + 4, :], nrow[:])  # B n-row
                    nc.sync.dma_start(U[a + 4:a + 5, :], nrow[:])  # A n-row
                    # A coords = -2 * B coords: scale in place (same base),
                    # then DMA into the A rows
                    nc.vector.tensor_scalar(
                        out=sq[b:b + 3, :], in0=U[b:b + 3, :],
                        scalar1=-2.0, scalar2=None,
                        op0=mybir.AluOpType.mult)
                    nc.sync.dma_start(U[a:a + 3, :], sq[b:b + 3, :])

            # ---- main passes ----
            with tc.tile_pool(name="rowbuf", bufs=2) as rowbuf_pool, \
                 tc.tile_pool(name="mainpsum", bufs=2, space="PSUM") as psum_pool, \
                 tc.tile_pool(name="scratch", bufs=2) as scratch_pool:
                for _ in range(repeat):
                    _emit_pass(nc, U1, U2, 0, d1t, i1t, rowbuf_pool, psum_pool,
                               scratch_pool, n_pts, m_pts, mm_dt, pool_folds)
                    _emit_pass(nc, U2, U1, 32, d2t, i2t, rowbuf_pool, psum_pool,
                               scratch_pool, m_pts, n_pts, mm_dt, pool_folds)

            # clamp tiny negative rounding like the reference's max(d, 0)
            nc.scalar.activation(d1t[:], d1t[:], mybir.ActivationFunctionType.Relu)
            nc.scalar.activation(d2t[:], d2t[:], mybir.ActivationFunctionType.Relu)
            nc.sync.dma_start(d1.ap(), d1t[:])
            nc.sync.dma_start(i1.ap(), i1t[:])
            nc.sync.dma_start(d2.ap(), d2t[:])
            nc.sync.dma_start(i2.ap(), i2t[:])

    nc.compile()
    _PROGRAM_CACHE[key] = nc
    return nc


def kernel(xyz1: np.ndarray, xyz2: np.ndarray, mm_dtype: str = "f32r",
           repeat: int = 1, _return_results_only: bool = False):
    xyz1 = np.asarray(xyz1, dtype=np.float32)
    xyz2 = np.asarray(xyz2, dtype=np.float32)
    B, N, _ = xyz1.shape
    _, M, _ = xyz2.shape
    assert B == 8 and N == 8192 and M == 8192, (B, N, M)

    nc = _build_program(N, M, B, mm_dtype, repeat)

    ones_n = np.ones((1, N), np.float32)
    ones_m = np.ones((1, M), np.float32)
    in_maps = [
        {"c1": np.concatenate([np.ascontiguousarray(xyz1[b].T), ones_n]),
         "c2": np.concatenate([np.ascontiguousarray(xyz2[b].T), ones_m])}
        for b in range(B)
    ]
    res = run_bass_kernel_spmd(nc, in_maps, list(range(B)))

    dist1 = np.empty((B, N), np.float32)
    dist2 = np.empty((B, M), np.float32)
    idx1 = np.empty((B, N), np.int32)
    idx2 = np.empty((B, M), np.int32)
    for b in range(B):
        r = res.results[b]
        dist1[b] = np.asarray(r["d1"]).T.reshape(-1)
        idx1[b] = np.asarray(r["i1"]).T.reshape(-1)
        dist2[b] = np.asarray(r["d2"]).T.reshape(-1)
        idx2[b] = np.asarray(r["i2"]).T.reshape(-1)
    return dist1, dist2, idx1, idx2

